# revision 54
# baseline (speedup 1.0000x reference)
"""Trainium2 Bass kernel for nn_Attention_8323646620215.

LayerNorm -> QKV -> scores(+rel-bias+mask) -> softmax -> attn@V -> out proj.

Sharding: 8 cores = (batch b in 0..3) x (query-half in 0..1). Each core
computes the full K/V for its batch and attention for its 1024 query rows;
no cross-core communication.

v2 design: one ACT-exp-bound pipeline. The softmax exp (128 x [128,1024]
f32 PSUM reads on the scalar engine) is the hard floor (~128us); everything
else is scheduled into the other engines' slack under it:

  - No on-chip transposes: the host also sends xT = x.T (bf16). Q/K/V are
    computed from RAW xT (un-normalized); LayerNorm enters algebraically:
      xn = (x - mu) * rstd  (gamma/beta folded into weights host-side)
      K_hat[out, tok] = rstd[tok] * (W.T @ xT - g (x) mu)[out, tok]
    The rank-1 mean term (g = colsum(W)) is one extra contraction-1 matmul
    accumulated into the same PSUM group; the per-token rstd scale is
    applied at PSUM evacuation (DVE tensor_tensor with a broadcast rstd_T
    tile built on-chip by a rank-1 PE matmul).
  - mu/rstd come from DVE bn_stats on the f32 x, shipped through a tiny
    DRAM roundtrip (gpsimd cast-DMA back as bf16 rows).
  - expA = exp(clip rel bias) * mask is precomputed host-side ([N, NQ]
    bf16, same bytes as the old mask DMA) - no Toeplitz build on-chip.
  - Phase C runs 8 passes (head-pair x query-half) x 16 key tiles:
    scores (2 row-packed 64-contraction matmuls) -> exp -> pb = eb*expA
    (stride-0 repeated read) -> attn@V (65-row stationary, ones row gives
    the softmax denominator). AV matmuls run LAG iterations behind the exp
    stream so V/K/Q production for later passes interleaves into PE slack.
  - Denominators: DRAM roundtrip reshape -> reciprocal -> broadcast read
    (as in v1), interleaved into the following pass.
"""
import sys
import types
import numpy as np

sys.path.insert(0, "/opt/trn_rl_repo")

# ---- environment fixes (axon agent container) -------------------------------
if "antenv.axon_hooks" not in sys.modules:
    _m = types.ModuleType("antenv.axon_hooks")
    _m._hook = None
    _m.set_axon_ntff_profile_hook = lambda h: setattr(_m, "_hook", h)
    _m.get_axon_ntff_profile_hook = lambda: _m._hook
    sys.modules["antenv.axon_hooks"] = _m
    try:
        from trn_agent_boot.trn_boot import _ntff_profile_via_ctypes
        _m._hook = _ntff_profile_via_ctypes("/opt/axon/libaxon_pjrt.so")
    except Exception:
        pass

import ml_dtypes  # noqa: E402
from concourse import bass, mybir, tile  # noqa: E402
from concourse.bass_utils import run_bass_kernel_spmd  # noqa: E402

F32 = mybir.dt.float32
BF16 = mybir.dt.bfloat16
AF = mybir.ActivationFunctionType
OP = mybir.AluOpType

B, N, D, H, DH, MAXREL = 4, 2048, 512, 8, 64, 200
NQ = N // 2          # queries per core
NT = N // 128        # 16 token tiles
NCORES = 8
LAG = 6              # AV matmuls trail the exp stream by this many iters

# This container's walrus rejects instructions with more than one sem wait.
# Splitting is sound: a same-engine NoOp right before the instruction
# enforces the wait at the same program point (sequencers run in order).


def _split_waits(nc, maxw=1):
    n_split = 0
    for f in nc.m.functions:
        for blk in f.blocks:
            bb = blk.bb if hasattr(blk, "bb") else blk
            insts = list(bb.instructions)
            out = []
            changed = False
            for inst in insts:
                si = inst.sync_info
                waits = list(si.on_wait) if si and si.on_wait else []
                if len(waits) > maxw:
                    extra = waits[:-maxw]
                    chunks = [extra[j:j + maxw] for j in range(0, len(extra), maxw)]
                    for i, chunk in enumerate(chunks):
                        nop = mybir.InstNoOp(name=f"{inst.name}-ws{i}", ins=[], outs=[])
                        nop.engine = inst.engine
                        nop.sync_info = mybir.SyncInfo(on_wait=chunk, on_update=[])
                        out.append(nop)
                    si.on_wait = waits[-maxw:]
                    changed = True
                    n_split += 1
                out.append(inst)
            if changed:
                bb.instructions = out
    return n_split


def build(has_c=False, has_b=False, split_waits=True):
    nc = bass.Bass("TRN2", target_bir_lowering=False, debug=False,
                   num_devices=NCORES)
    x_d = nc.dram_tensor("x", [N, D], BF16, kind="ExternalInput")
    xt_d = nc.dram_tensor("xt", [D, N], BF16, kind="ExternalInput")
    wqkv_d = nc.dram_tensor("wqkv", [D, 3 * D], BF16, kind="ExternalInput")
    grow_d = nc.dram_tensor("grow", [1, 3 * D], BF16, kind="ExternalInput")
    wout_d = nc.dram_tensor("wout", [D, D], BF16, kind="ExternalInput")
    bout_d = nc.dram_tensor("bout", [D], F32, kind="ExternalInput")
    cqkv_d = nc.dram_tensor("cqkv", [3 * D], F32, kind="ExternalInput")
    expa_d = nc.dram_tensor("expa", [N, NQ], BF16, kind="ExternalInput")
    mu_d = nc.dram_tensor("mu_scratch", [1, N], F32)
    rs_d = nc.dram_tensor("rs_scratch", [1, N], F32)
    dsb_d = nc.dram_tensor("den_scratch", [H, NQ], BF16)
    dsi_d = nc.dram_tensor("invden_scratch", [H, NQ], BF16)
    y_d = nc.dram_tensor("y", [NQ, D], F32, kind="ExternalOutput")

    with tile.TileContext(nc) as tc, \
         tc.tile_pool(name="const", bufs=1) as C, \
         tc.tile_pool(name="pers", bufs=1) as P, \
         tc.tile_pool(name="work", bufs=3) as W:

        # ---- persistent tiles ----------------------------------------------
        # Every DMA-written tile is written by exactly ONE dma_start (Tile's
        # write-hazard tracking is coarse; multi-DMA tiles make any consumer
        # wait for the LAST write to the tile).
        xt_ch = [[P.tile([128, 512], BF16, tag=f"xt{fb}_{c}",
                         name=f"xt{fb}_{c}") for c in range(4)]
                 for fb in range(4)]
        KTp = [P.tile([128, N], BF16, tag=f"KT{hp}", name=f"KT{hp}") for hp in range(4)]
        QTp = [P.tile([128, NQ], BF16, tag=f"QT{hp}", name=f"QT{hp}") for hp in range(4)]
        Vau = [P.tile([128, H, 66], BF16, tag=f"V{t}", name=f"Vau{t}") for t in range(NT)]
        expa_q = [P.tile([128, 4, NQ], BF16, tag=f"eA{q}", name=f"eA{q}")
                  for q in range(4)]
        numT = [P.tile([65, NQ], BF16, tag=f"nT{h}", name=f"nT{h}") for h in range(H)]
        pairT = [P.tile([128, NQ], BF16, tag=f"pT{hp}", name=f"pT{hp}") for hp in range(4)]
        rstdT = P.tile([128, N], BF16, tag="rstdT", name="rstdT")
        mu_cc = [P.tile([128, 4], F32, tag=f"mu{c}", name=f"mu{c}")
                 for c in range(4)]
        rs_cc = [P.tile([128, 4], F32, tag=f"rs{c}", name=f"rs{c}")
                 for c in range(4)]
        murow_c = [P.tile([1, 512], BF16, tag=f"mur{c}", name=f"mur{c}")
                   for c in range(4)]
        rsrow_c = [P.tile([1, 512], BF16, tag=f"rsr{c}", name=f"rsr{c}")
                   for c in range(4)]
        rs_cols = [P.tile([128, 4], F32, tag=f"rsc{c}", name=f"rsc{c}")
                   for c in range(4)]

        # ---- DMA issue plan ------------------------------------------------
        # Each dma_start costs ~600ns on its issuing engine and queue
        # bandwidth depends on descriptor size (contiguous run length), so:
        # x is partition-remapped so each partition holds 4 consecutive
        # DRAM rows (4KB descriptors), queues are load-balanced and ordered
        # by consumer deadline, and wqkv columns are host-reordered to
        # [K0 Q0 K1 Q1 K2 Q2 K3 Q3 V].
        # x_ch[c][p, t, f] = x[512c + 4p + t, f]  (token = 512c + 4p + t)
        x_ch = [P.tile([128, 4, D], BF16, tag=f"xch{c}", name=f"xch{c}")
                for c in range(4)]

        def x_dma(eng, c):
            eng.dma_start(
                out=x_ch[c][:],
                in_=bass.AP(tensor=x_d.ap().tensor, offset=c * 512 * D,
                            ap=[[4 * D, 128], [D, 4], [1, D]]))

        def xt_dma(eng, fb, c):
            eng.dma_start(
                out=xt_ch[fb][c][:],
                in_=xt_d[fb * 128:(fb + 1) * 128, c * 512:(c + 1) * 512])

        def expa_dma(eng, q):
            eng.dma_start(
                out=expa_q[q][:],
                in_=bass.AP(tensor=expa_d.ap().tensor, offset=q * 512 * NQ,
                            ap=[[NQ, 128], [128 * NQ, 4], [1, NQ]]))

        wqkv_sb = [C.tile([128, 3 * D], BF16, tag=f"wq{kb}", name=f"wq{kb}")
                   for kb in range(4)]
        woutP = [C.tile([128, D], BF16, tag=f"woutP{hp}", name=f"woutP{hp}")
                 for hp in range(4)]
        grow_sb = C.tile([1, 3 * D], BF16, tag="grow")

        # scalar: wqkv kb2/kb3 + xt fb0/fb1 chunks 0-1 (only ~1.1 MB; the
        # sqrts behind these 4 issues still run on time).
        for kb in range(2, 4):
            nc.scalar.dma_start(out=wqkv_sb[kb][:],
                                in_=wqkv_d[kb * 128:(kb + 1) * 128, :])
        for c in range(2):
            xt_dma(nc.scalar, 0, c)
            xt_dma(nc.scalar, 1, c)
        # sync: all of x (4KB descriptors, fast), wqkv kb0/kb1, xt fb0/fb1
        # chunks 2-3, then parked expA q1-3; den/y traffic comes later.
        for c in range(4):
            x_dma(nc.sync, c)
        for kb in range(2):
            nc.sync.dma_start(out=wqkv_sb[kb][:],
                              in_=wqkv_d[kb * 128:(kb + 1) * 128, :])
        for c in range(2, 4):
            xt_dma(nc.sync, 0, c)
            xt_dma(nc.sync, 1, c)
        for q in range(1, 4):
            expa_dma(nc.sync, q)
        # gpsimd: xt fb2/fb3 chunks 0-1, grow; then the mu/rs roundtrips
        # (low latency: nothing bulky ahead), expA q0, xt fb2/fb3 rest.
        for c in range(2):
            xt_dma(nc.gpsimd, 2, c)
            xt_dma(nc.gpsimd, 3, c)
        nc.gpsimd.dma_start(out=grow_sb[:], in_=grow_d[0:1, :])

        if has_c:
            # cqkv columns are host-reordered the same way as wqkv
            cq_all = C.tile([128, 12], F32, tag="cq")
            nc.gpsimd.dma_start(
                out=cq_all[:],
                in_=bass.AP(tensor=cqkv_d.ap().tensor, offset=0,
                            ap=[[1, 128], [128, 12]]))
            cv_bc = C.tile([128, D], F32, tag="cv")
            nc.gpsimd.dma_start(
                out=cv_bc[:],
                in_=bass.AP(tensor=cqkv_d.ap().tensor, offset=2 * D,
                            ap=[[0, 128], [1, D]]))
        if has_b:
            bout_bc = C.tile([128, D], F32, tag="bout")
            nc.gpsimd.dma_start(
                out=bout_bc[:],
                in_=bass.AP(tensor=bout_d.ap().tensor, offset=0,
                            ap=[[0, 128], [1, D]]))

        def late_gpsimd_consts():
            for hp in range(4):
                nc.gpsimd.dma_start(out=woutP[hp][:],
                                    in_=wout_d[hp * 128:(hp + 1) * 128, :])

        ones1 = C.tile([1, 128], BF16, tag="ones1")
        nc.vector.memset(ones1[:], 1.0)
        eps_t = C.tile([128, 1], F32, tag="eps")
        nc.vector.memset(eps_t[:], 1e-5)

        # mu/rs roundtrip per 4-tile chunk: out on sync (f32, token order
        # mu_d[512c + 4p + t] <- mu_cc[c][p, t]), back on gpsimd (cast to
        # bf16 rows) + an f32 per-tile column view for the V evacuation.
        # All must be EMITTED after the stats that write mu_cc/rs_cc
        # (program order is logical order in Tile); backs are emitted
        # separately so they don't head-block the gpsimd queue.
        def rt_out(c, eng=None):
            eng = eng or nc.gpsimd
            sl_s = [[4, 128], [1, 4]]
            eng.dma_start(
                out=bass.AP(tensor=mu_d.ap().tensor, offset=512 * c, ap=sl_s),
                in_=mu_cc[c][:])
            eng.dma_start(
                out=bass.AP(tensor=rs_d.ap().tensor, offset=512 * c, ap=sl_s),
                in_=rs_cc[c][:])

        def rt_back(c, eng=None):
            eng = eng or nc.gpsimd
            eng.dma_start(out=murow_c[c][:],
                          in_=mu_d[0:1, 512 * c:512 * (c + 1)])
            eng.dma_start(out=rsrow_c[c][:],
                          in_=rs_d[0:1, 512 * c:512 * (c + 1)])
            nc.gpsimd.dma_start(
                out=rs_cols[c][:],
                in_=bass.AP(tensor=rs_d.ap().tensor, offset=512 * c,
                            ap=[[1, 128], [128, 4]]))

        # ---- LayerNorm stats (DVE + a gpsimd rsqrt; the ACT engine and
        # its DMA-ring-backpressured queue stay out of the critical chain) --
        # stats slot (c, s) covers tokens {512c + 4p + s : p in 0..127}
        def ln_stats(t):
            c, s = t // 4, t % 4
            st = W.tile([128, 6], F32, tag="st")
            nc.vector.bn_stats(out=st[:], in_=x_ch[c][:, s, :])
            mv = W.tile([128, 2], F32, tag="mv", bufs=4, name=f"mv{t}")
            nc.vector.bn_aggr(out=mv[:], in_=st[:])
            nc.vector.tensor_copy(out=mu_cc[c][:, s:s + 1], in_=mv[:, 0:1])
            rsq = W.tile([128, 1], F32, tag="rsq", bufs=4, name=f"rsq{t}")
            nc.scalar.activation(out=rsq[:], in_=mv[:, 1:2], func=AF.Sqrt,
                                 bias=eps_t[:])
            nc.vector.reciprocal(out=rs_cc[c][:, s:s + 1], in_=rsq[:])

        # ---- B-work closures (PE production of rstdT / K / Q / V) ----------
        with tc.tile_pool(name="psB", bufs=1, space="PSUM") as psB, \
             tc.tile_pool(name="psC", bufs=1, space="PSUM") as psC:

            def wkq(kb, hp, is_q):
                off = 256 * hp + (128 if is_q else 0)
                return wqkv_sb[kb][:, off:off + 128]

            def rstdT_build(c, on_act=False):
                def f():
                    bp = psB.tile([128, 512], F32, tag="bps", bufs=2)
                    nc.tensor.matmul(bp[:], ones1[:], rsrow_c[c][:],
                                     start=True, stop=True)
                    if on_act:
                        nc.scalar.copy(
                            out=rstdT[:, c * 512:(c + 1) * 512], in_=bp[:])
                    else:
                        nc.vector.tensor_copy(
                            out=rstdT[:, c * 512:(c + 1) * 512], in_=bp[:])
                return [f]

            def k_chunk(hp, c):
                """KTp[hp][:, c*512:(c+1)*512] (both heads' 128 rows)."""
                def mms():
                    kp = psB.tile([128, 512], F32, tag="bps", bufs=2)
                    for kb in range(4):
                        nc.tensor.matmul(
                            kp[:], wkq(kb, hp, False), xt_ch[kb][c][:],
                            start=(kb == 0), stop=False)
                    nc.tensor.matmul(
                        kp[:], grow_sb[0:1, 256 * hp:256 * hp + 128],
                        murow_c[c][:], start=False, stop=True)
                    sl = slice(c * 512, (c + 1) * 512)
                    nc.vector.tensor_mul(out=KTp[hp][:, sl], in0=kp[:],
                                         in1=rstdT[:, sl])
                    if has_c:
                        nc.vector.tensor_scalar_add(
                            out=KTp[hp][:, sl], in0=KTp[hp][:, sl],
                            scalar1=cq_all[:, 2 * hp:2 * hp + 1])
                return [mms]

            def q_chunk(hp, ic):
                """QTp[hp][:, ic*512:(ic+1)*512] (queries = tokens ic-half)."""
                def mms():
                    qp = psB.tile([128, 512], F32, tag="bps", bufs=2)
                    for kb in range(4):
                        nc.tensor.matmul(
                            qp[:], wkq(kb, hp, True), xt_ch[kb][ic][:],
                            start=(kb == 0), stop=False)
                    nc.tensor.matmul(
                        qp[:], grow_sb[0:1, 256 * hp + 128:256 * (hp + 1)],
                        murow_c[ic][:], start=False, stop=True)
                    sl = slice(ic * 512, (ic + 1) * 512)
                    nc.vector.tensor_mul(out=QTp[hp][:, sl], in0=qp[:],
                                         in1=rstdT[:, sl])
                    if has_c:
                        nc.vector.tensor_scalar_add(
                            out=QTp[hp][:, sl], in0=QTp[hp][:, sl],
                            scalar1=cq_all[:, 2 * hp + 1:2 * hp + 2])
                return [mms]

            def v_hp(t, hp):
                """Vau[t][:, 2hp:2hp+2, 0:64] = rstd * (x @ Wv_hp - mu (x) g);
                pass (hp, ic) only needs its own pair's V columns, so V
                production spreads evenly across the passes (128-col mms)."""
                def mms():
                    c, s = t // 4, t % 4
                    vp = psB.tile([128, 512], F32, tag="bps", bufs=2)
                    vsl = slice(1024 + 128 * hp, 1024 + 128 * (hp + 1))
                    for kb in range(4):
                        nc.tensor.matmul(
                            vp[:, 0:128],
                            xt_ch[kb][c][:, 128 * s:128 * (s + 1)],
                            wqkv_sb[kb][:, vsl],
                            start=(kb == 0), stop=False)
                    nc.tensor.matmul(
                        vp[:, 0:128], murow_c[c][0:1, 128 * s:128 * (s + 1)],
                        grow_sb[0:1, 2 * D + 128 * hp:2 * D + 128 * (hp + 1)],
                        start=False, stop=True)
                    if hp == 0:
                        nc.vector.memset(Vau[t][:, :, 64:65], 1.0)
                    nc.vector.tensor_scalar_mul(
                        out=Vau[t][:, 2 * hp:2 * hp + 2, 0:64],
                        in0=vp[:, 0:128], scalar1=rs_cols[c][:, s:s + 1])
                    if has_c:
                        nc.vector.tensor_add(
                            out=Vau[t][:, 2 * hp:2 * hp + 2, 0:64],
                            in0=Vau[t][:, 2 * hp:2 * hp + 2, 0:64],
                            in1=cv_bc[:, 128 * hp:128 * (hp + 1)])
                return [mms]

            # ---- denominator pipeline (unchanged from v1) ------------------
            def den_pieces(hp):
                dal = W.tile([128, 2 * NQ // 128], BF16, tag="dall", bufs=2,
                             name=f"dal{hp}")
                dbs = [None, None]

                def p0():
                    nc.sync.dma_start(
                        out=dal[:],
                        in_=bass.AP(tensor=dsb_d.ap().tensor, offset=2 * hp * NQ,
                                    ap=[[2 * NQ // 128, 128], [1, 2 * NQ // 128]]))

                def p1():
                    nc.vector.tensor_scalar_add(out=dal[:], in0=dal[:],
                                                scalar1=1e-20)
                    with nc.allow_low_precision(reason="bf16 softmax denominators"):
                        nc.vector.reciprocal(out=dal[:], in_=dal[:])
                    nc.sync.dma_start(
                        out=bass.AP(tensor=dsi_d.ap().tensor, offset=2 * hp * NQ,
                                    ap=[[2 * NQ // 128, 128], [1, 2 * NQ // 128]]),
                        in_=dal[:])

                def load_bc(e):
                    def f():
                        h = 2 * hp + e
                        den_bc = W.tile([64, NQ], BF16, tag="denb", bufs=2,
                                        name=f"denb{h}")
                        dbs[e] = den_bc
                        nc.sync.dma_start(
                            out=den_bc[:],
                            in_=bass.AP(tensor=dsi_d.ap().tensor, offset=h * NQ,
                                        ap=[[0, 64], [1, NQ]]))
                    return f

                def mul_chunk(e, half):
                    def f():
                        h = 2 * hp + e
                        sl = slice(half * 512, (half + 1) * 512)
                        if e == 0:
                            nc.vector.tensor_mul(out=pairT[hp][0:64, sl],
                                                 in0=numT[h][0:64, sl],
                                                 in1=dbs[e][:, sl])
                        else:
                            nc.vector.tensor_mul(out=numT[h][0:64, sl],
                                                 in0=numT[h][0:64, sl],
                                                 in1=dbs[e][:, sl])
                    return f

                def stitch():
                    nc.sync.dma_start(out=pairT[hp][64:128, :],
                                      in_=numT[2 * hp + 1][0:64, :])

                return [p0, None, None, None, p1, None, load_bc(0),
                        load_bc(1), None, None, mul_chunk(0, 0),
                        mul_chunk(0, 1), mul_chunk(1, 0), mul_chunk(1, 1),
                        stitch]

            # ---- prologue: stats waves + roundtrips + parked bulk DMA ------
            for t in range(4):
                ln_stats(t)
            rt_out(0)
            rt_back(0)
            for t in range(4, 8):
                ln_stats(t)
            rt_out(1)
            rt_back(1)
            expa_dma(nc.gpsimd, 0)
            xt_dma(nc.gpsimd, 2, 2)
            xt_dma(nc.gpsimd, 3, 2)
            xt_dma(nc.gpsimd, 2, 3)
            xt_dma(nc.gpsimd, 3, 3)
            for t in range(8, 12):
                ln_stats(t)
            rt_out(2)
            rt_back(2)

            # PE warm-up (HAM) while waiting for the mu/rs roundtrip, then
            # everything iter0 needs: rstdT c0, K[0,0], Q[0,0].
            for wu in range(30):
                dmy = psB.tile([128, 512], F32, tag="bps", bufs=2,
                               name=f"wu{wu}")
                nc.tensor.matmul(dmy[:], xt_ch[2][0][:, 0:128], xt_ch[2][0][:],
                                 start=True, stop=True)
            rstdT_build(0, on_act=True)[0]()
            k_chunk(0, 0)[0]()
            q_chunk(0, 0)[0]()

            # Remaining B-work, popped 2/iter inside C. Order respects
            # both data deadlines and global program-order hazards
            # (producers must be emitted before their consumers).
            workq = [lambda t=t: ln_stats(t) for t in range(12, NT)]
            workq += [lambda: (rt_out(3, nc.gpsimd), rt_back(3),
                               late_gpsimd_consts())]
            workq += rstdT_build(1)
            workq += k_chunk(0, 1)
            workq += v_hp(0, 0)
            workq += v_hp(1, 0)
            workq += rstdT_build(2)
            workq += k_chunk(0, 2)
            workq += v_hp(2, 0)
            workq += v_hp(3, 0)
            workq += rstdT_build(3)
            workq += k_chunk(0, 3)
            workq += v_hp(4, 0)
            workq += v_hp(5, 0)
            workq += v_hp(6, 0)
            workq += q_chunk(0, 1)
            for t in range(7, NT):
                workq += v_hp(t, 0)
            # production for later passes: pass p uses K[p//2], Q[p//2, p%2]
            # and V columns of pair p//2; all of it drains a pass ahead.
            for hp in range(1, 4):
                for c in range(4):
                    workq += k_chunk(hp, c)
                workq += q_chunk(hp, 0)
                workq += q_chunk(hp, 1)
                for t in range(NT):
                    workq += v_hp(t, hp)

            # ---- Phase C: 8 passes x 16 key tiles --------------------------
            avq = []   # (closure) AV work trailing the exp stream

            def av_mms(p, jt, pbt, av0, av1):
                def f():
                    hp = p // 2
                    h0, h1 = 2 * hp, 2 * hp + 1
                    nc.tensor.matmul(av0[:], Vau[jt][:, h0, 0:65],
                                     pbt[:, 0:512],
                                     start=(jt == 0), stop=(jt == NT - 1))
                    nc.tensor.matmul(av1[:], Vau[jt][:, h1, 0:65],
                                     pbt[:, 512:1024],
                                     start=(jt == 0), stop=(jt == NT - 1))
                return f

            for p in range(8):
                hp, ic = p // 2, p % 2
                av0 = psC.tile([65, 512], F32, tag="av0", bufs=1,
                               name=f"av0_{p}")
                av1 = psC.tile([65, 512], F32, tag="av1", bufs=1,
                               name=f"av1_{p}")
                i5 = ic * 512
                for jt in range(NT):
                    sp = psC.tile([128, 1024], F32, tag="sp", bufs=2)
                    nc.tensor.matmul(
                        sp[:, 0:512],
                        KTp[hp][0:64, jt * 128:(jt + 1) * 128],
                        QTp[hp][0:64, i5:i5 + 512],
                        start=True, stop=True, tile_position=(0, 0))
                    nc.tensor.matmul(
                        sp[:, 512:1024],
                        KTp[hp][64:128, jt * 128:(jt + 1) * 128],
                        QTp[hp][64:128, i5:i5 + 512],
                        start=True, stop=True, tile_position=(64, 0))
                    eb = W.tile([128, 1024], BF16, tag="eb", bufs=3)
                    nc.scalar.activation(out=eb[:], in_=sp[:], func=AF.Exp)
                    pbt = W.tile([128, 1024], BF16, tag="pb", bufs=LAG + 2)
                    ea = expa_q[jt // 4][:, jt % 4, i5:i5 + 512]
                    nc.vector.tensor_mul(out=pbt[:, 0:512], in0=eb[:, 0:512],
                                         in1=ea)
                    nc.vector.tensor_mul(out=pbt[:, 512:1024],
                                         in0=eb[:, 512:1024], in1=ea)
                    avq.append(av_mms(p, jt, pbt, av0, av1))
                    # drain trailing B-work (2/iter keeps producers ahead
                    # of their PE-queue consumers), then lagged AV work
                    # (also at most 2/iter so den roundtrips stay spread).
                    for _ in range(2):
                        if workq:
                            workq.pop(0)()
                    for _ in range(2):
                        if len(avq) > LAG:
                            fn = avq.pop(0)
                            if fn is not None:
                                fn()
                # end of pass: trailing AV for this pass still in avq; queue
                # the evacuation work behind them.
                def pass_tail(p=p, av0=av0, av1=av1):
                    hp, ic = p // 2, p % 2
                    h0, h1 = 2 * hp, 2 * hp + 1
                    sl = slice(ic * 512, (ic + 1) * 512)
                    def f():
                        nc.vector.tensor_copy(out=numT[h0][:, sl], in_=av0[:])
                        nc.vector.tensor_copy(out=numT[h1][:, sl], in_=av1[:])
                        if ic == 1:
                            for e in range(2):
                                h = 2 * hp + e
                                nc.sync.dma_start(out=dsb_d[h, :],
                                                  in_=numT[h][64:65, :])
                    return f
                avq.append(pass_tail())
                if p % 2 == 1:
                    avq.extend(den_pieces(p // 2))

            # flush remaining trailing work, keeping the PE busy through the
            # last denominator roundtrip (dummy matmuls bridge HAM warmth)
            tail = [fn for fn in avq if fn is not None]
            for i, fn in enumerate(tail):
                fn()
                if i % 2 == 1:
                    dmy = psB.tile([128, 512], F32, tag="bps", bufs=2,
                                   name=f"dmy{i}")
                    nc.tensor.matmul(dmy[:], wqkv_sb[0][:, 0:128],
                                     xt_ch[0][0][:], start=True, stop=True)

        # ---- Phase D: output projection (head pairs, K=128) ----------------
        with tc.tile_pool(name="psD", bufs=1, space="PSUM") as psD:
            yps = [psD.tile([128, 512], F32, tag=f"yp{isl}", name=f"yp{isl}")
                   for isl in range(8)]
            for hp in range(4):
                for isl in range(8):
                    nc.tensor.matmul(yps[isl][:],
                                     pairT[hp][:, isl * 128:(isl + 1) * 128],
                                     woutP[hp][:],
                                     start=(hp == 0), stop=(hp == 3))
            for isl in range(8):
                ysb = W.tile([128, 512], F32, tag="ysb", bufs=2)
                if has_b:
                    nc.vector.tensor_add(out=ysb[:], in0=yps[isl][:],
                                         in1=bout_bc[:])
                elif isl % 2 == 0:
                    nc.vector.tensor_copy(out=ysb[:], in_=yps[isl][:])
                else:
                    nc.scalar.copy(out=ysb[:], in_=yps[isl][:])
                nc.sync.dma_start(out=y_d[isl * 128:(isl + 1) * 128, :],
                                  in_=ysb[:])
    if split_waits:
        _split_waits(nc)
    return nc


_NC_CACHE = {}


def _get_nc(has_c, has_b):
    key = (has_c, has_b)
    if key not in _NC_CACHE:
        _NC_CACHE[key] = build(has_c, has_b)
    return _NC_CACHE[key]


LAST_EXEC_TIME_NS = None


def kernel(x, gamma, beta, Wqkv, Wout, bout, rel_table, temporal_mask,
           trace=True):
    global LAST_EXEC_TIME_NS
    x = np.asarray(x, np.float32)
    gamma = np.asarray(gamma, np.float32)
    beta = np.asarray(beta, np.float32)
    Wqkv = np.asarray(Wqkv, np.float32)
    Wout = np.asarray(Wout, np.float32)
    bout = np.asarray(bout, np.float32)
    rel_table = np.asarray(rel_table, np.float32)
    temporal_mask = np.asarray(temporal_mask)

    scale = DH ** -0.5
    w_eff = (Wqkv * gamma[:, None]).copy()
    w_eff[:, :D] *= scale
    cqkv = (beta @ Wqkv).astype(np.float32)
    cqkv[:D] *= scale
    # column reorder: [K0 Q0 K1 Q1 K2 Q2 K3 Q3 V] so hp0's K/Q land first
    perm = []
    for hp in range(H // 2):
        perm += list(range(D + hp * 128, D + (hp + 1) * 128))
        perm += list(range(hp * 128, (hp + 1) * 128))
    perm += list(range(2 * D, 3 * D))
    w_eff = np.ascontiguousarray(w_eff[:, perm])
    cqkv = np.ascontiguousarray(cqkv[perm])
    wqkv_bf = w_eff.astype(ml_dtypes.bfloat16)
    # rank-1 mean correction: -g rows (so the matmul accumulates -mu*g)
    grow_bf = (-w_eff.sum(axis=0)).reshape(1, -1).astype(ml_dtypes.bfloat16)
    wout_bf = Wout.astype(ml_dtypes.bfloat16)
    mask01 = (temporal_mask != 0)

    idx = np.arange(N)
    # expbias[i, j] = exp(rel_table[clip(i - j)]) with i=query, j=key
    expbias = np.exp(rel_table[
        np.clip(idx[:, None] - idx[None, :], -(MAXREL - 1), MAXREL - 1)
        + MAXREL - 1]).astype(np.float32)

    keyperm_half = [
        np.concatenate([np.arange(i0, i0 + NQ),
                        np.arange(NQ - i0, NQ - i0 + NQ)])
        for i0 in (0, NQ)
    ]
    # expA[j_perm, i_local] = exp(bias(query i, key j)) * mask(query i, key j)
    expa_half = []
    for half in range(2):
        kp = keyperm_half[half]
        qs = np.arange(half * NQ, (half + 1) * NQ)
        a = (expbias[np.ix_(qs, kp)] * mask01[np.ix_(qs, kp)]).T
        expa_half.append(np.ascontiguousarray(a).astype(ml_dtypes.bfloat16))

    in_maps = []
    for c in range(NCORES):
        b, half = c // 2, c % 2
        xp = np.ascontiguousarray(x[b][keyperm_half[half]])
        xtp = np.ascontiguousarray(xp.T).astype(ml_dtypes.bfloat16)
        in_maps.append({
            "x": xp.astype(ml_dtypes.bfloat16),
            "xt": xtp,
            "wqkv": wqkv_bf,
            "grow": grow_bf,
            "cqkv": cqkv,
            "wout": wout_bf,
            "bout": bout,
            "expa": expa_half[half],
        })

    nc = _get_nc(bool(np.any(cqkv != 0.0)), bool(np.any(bout != 0.0)))
    res = run_bass_kernel_spmd(nc, in_maps, core_ids=list(range(NCORES)),
                               trace=trace)
    LAST_EXEC_TIME_NS = res.exec_time_ns

    out = np.empty((B, N, D), np.float32)
    for c in range(NCORES):
        b, half = c // 2, c % 2
        out[b, half * NQ:(half + 1) * NQ] = res.results[c]["y"]
    return out


# revision 55
# speedup vs baseline: 1.0214x; 1.0214x over previous
"""Trainium2 Bass kernel for nn_Attention_8323646620215.

LayerNorm -> QKV -> scores(+rel-bias+mask) -> softmax -> attn@V -> out proj.

Sharding: 8 cores = (batch b in 0..3) x (query-half in 0..1). Each core
computes the full K/V for its batch and attention for its 1024 query rows;
no cross-core communication.

v2 design: one ACT-exp-bound pipeline. The softmax exp (128 x [128,1024]
f32 PSUM reads on the scalar engine) is the hard floor (~128us); everything
else is scheduled into the other engines' slack under it:

  - No on-chip transposes: the host also sends xT = x.T (bf16). Q/K/V are
    computed from RAW xT (un-normalized); LayerNorm enters algebraically:
      xn = (x - mu) * rstd  (gamma/beta folded into weights host-side)
      K_hat[out, tok] = rstd[tok] * (W.T @ xT - g (x) mu)[out, tok]
    The rank-1 mean term (g = colsum(W)) is one extra contraction-1 matmul
    accumulated into the same PSUM group; the per-token rstd scale is
    applied at PSUM evacuation (DVE tensor_tensor with a broadcast rstd_T
    tile built on-chip by a rank-1 PE matmul).
  - mu/rstd come from DVE bn_stats on the f32 x, shipped through a tiny
    DRAM roundtrip (gpsimd cast-DMA back as bf16 rows).
  - expA = exp(clip rel bias) * mask is precomputed host-side ([N, NQ]
    bf16, same bytes as the old mask DMA) - no Toeplitz build on-chip.
  - Phase C runs 8 passes (head-pair x query-half) x 16 key tiles:
    scores (2 row-packed 64-contraction matmuls) -> exp -> pb = eb*expA
    (stride-0 repeated read) -> attn@V (65-row stationary, ones row gives
    the softmax denominator). AV matmuls run LAG iterations behind the exp
    stream so V/K/Q production for later passes interleaves into PE slack.
  - Denominators: DRAM roundtrip reshape -> reciprocal -> broadcast read
    (as in v1), interleaved into the following pass.
"""
import sys
import types
import numpy as np

sys.path.insert(0, "/opt/trn_rl_repo")

# ---- environment fixes (axon agent container) -------------------------------
if "antenv.axon_hooks" not in sys.modules:
    _m = types.ModuleType("antenv.axon_hooks")
    _m._hook = None
    _m.set_axon_ntff_profile_hook = lambda h: setattr(_m, "_hook", h)
    _m.get_axon_ntff_profile_hook = lambda: _m._hook
    sys.modules["antenv.axon_hooks"] = _m
    try:
        from trn_agent_boot.trn_boot import _ntff_profile_via_ctypes
        _m._hook = _ntff_profile_via_ctypes("/opt/axon/libaxon_pjrt.so")
    except Exception:
        pass

import ml_dtypes  # noqa: E402
from concourse import bass, mybir, tile  # noqa: E402
from concourse.bass_utils import run_bass_kernel_spmd  # noqa: E402

F32 = mybir.dt.float32
BF16 = mybir.dt.bfloat16
AF = mybir.ActivationFunctionType
OP = mybir.AluOpType

B, N, D, H, DH, MAXREL = 4, 2048, 512, 8, 64, 200
NQ = N // 2          # queries per core
NT = N // 128        # 16 token tiles
NCORES = 8
LAG = 6              # AV matmuls trail the exp stream by this many iters

# This container's walrus rejects instructions with more than one sem wait.
# Splitting is sound: a same-engine NoOp right before the instruction
# enforces the wait at the same program point (sequencers run in order).


def _split_waits(nc, maxw=1):
    n_split = 0
    for f in nc.m.functions:
        for blk in f.blocks:
            bb = blk.bb if hasattr(blk, "bb") else blk
            insts = list(bb.instructions)
            out = []
            changed = False
            for inst in insts:
                si = inst.sync_info
                waits = list(si.on_wait) if si and si.on_wait else []
                if len(waits) > maxw:
                    extra = waits[:-maxw]
                    chunks = [extra[j:j + maxw] for j in range(0, len(extra), maxw)]
                    for i, chunk in enumerate(chunks):
                        nop = mybir.InstNoOp(name=f"{inst.name}-ws{i}", ins=[], outs=[])
                        nop.engine = inst.engine
                        nop.sync_info = mybir.SyncInfo(on_wait=chunk, on_update=[])
                        out.append(nop)
                    si.on_wait = waits[-maxw:]
                    changed = True
                    n_split += 1
                out.append(inst)
            if changed:
                bb.instructions = out
    return n_split


def build(has_c=False, has_b=False, split_waits=True):
    nc = bass.Bass("TRN2", target_bir_lowering=False, debug=False,
                   num_devices=NCORES)
    x_d = nc.dram_tensor("x", [N, D], BF16, kind="ExternalInput")
    xt_d = nc.dram_tensor("xt", [D, N], BF16, kind="ExternalInput")
    wqkv_d = nc.dram_tensor("wqkv", [D, 3 * D], BF16, kind="ExternalInput")
    grow_d = nc.dram_tensor("grow", [1, 3 * D], BF16, kind="ExternalInput")
    wout_d = nc.dram_tensor("wout", [D, D], BF16, kind="ExternalInput")
    bout_d = nc.dram_tensor("bout", [D], F32, kind="ExternalInput")
    cqkv_d = nc.dram_tensor("cqkv", [3 * D], F32, kind="ExternalInput")
    expa_d = nc.dram_tensor("expa", [N, NQ], BF16, kind="ExternalInput")
    mu_d = nc.dram_tensor("mu_scratch", [1, N], F32)
    rs_d = nc.dram_tensor("rs_scratch", [1, N], F32)
    dsb_d = nc.dram_tensor("den_scratch", [H, NQ], BF16)
    dsi_d = nc.dram_tensor("invden_scratch", [H, NQ], BF16)
    y_d = nc.dram_tensor("y", [NQ, D], F32, kind="ExternalOutput")

    with tile.TileContext(nc) as tc, \
         tc.tile_pool(name="const", bufs=1) as C, \
         tc.tile_pool(name="pers", bufs=1) as P, \
         tc.tile_pool(name="work", bufs=3) as W:

        # ---- persistent tiles ----------------------------------------------
        # Every DMA-written tile is written by exactly ONE dma_start (Tile's
        # write-hazard tracking is coarse; multi-DMA tiles make any consumer
        # wait for the LAST write to the tile).
        xt_ch = [[P.tile([128, 512], BF16, tag=f"xt{fb}_{c}",
                         name=f"xt{fb}_{c}") for c in range(4)]
                 for fb in range(4)]
        KTp = [P.tile([128, N], BF16, tag=f"KT{hp}", name=f"KT{hp}") for hp in range(4)]
        QTp = [P.tile([128, NQ], BF16, tag=f"QT{hp}", name=f"QT{hp}") for hp in range(4)]
        Vau = [P.tile([128, H, 66], BF16, tag=f"V{t}", name=f"Vau{t}") for t in range(NT)]
        expa_q = [P.tile([128, 4, NQ], BF16, tag=f"eA{q}", name=f"eA{q}")
                  for q in range(4)]
        numT = [P.tile([65, NQ], BF16, tag=f"nT{h}", name=f"nT{h}") for h in range(H)]
        pairT = [P.tile([128, NQ], BF16, tag=f"pT{hp}", name=f"pT{hp}") for hp in range(4)]
        rstdT = P.tile([128, N], BF16, tag="rstdT", name="rstdT")
        mu_cc = [P.tile([128, 4], F32, tag=f"mu{c}", name=f"mu{c}")
                 for c in range(4)]
        rs_cc = [P.tile([128, 4], F32, tag=f"rs{c}", name=f"rs{c}")
                 for c in range(4)]
        murow_c = [P.tile([1, 512], BF16, tag=f"mur{c}", name=f"mur{c}")
                   for c in range(4)]
        rsrow_c = [P.tile([1, 512], BF16, tag=f"rsr{c}", name=f"rsr{c}")
                   for c in range(4)]
        rs_cols = [P.tile([128, 4], F32, tag=f"rsc{c}", name=f"rsc{c}")
                   for c in range(4)]

        # ---- DMA issue plan ------------------------------------------------
        # Each dma_start costs ~600ns on its issuing engine and queue
        # bandwidth depends on descriptor size (contiguous run length), so:
        # x is partition-remapped so each partition holds 4 consecutive
        # DRAM rows (4KB descriptors), queues are load-balanced and ordered
        # by consumer deadline, and wqkv columns are host-reordered to
        # [K0 Q0 K1 Q1 K2 Q2 K3 Q3 V].
        # x_ch[c][p, t, f] = x[512c + 4p + t, f]  (token = 512c + 4p + t)
        x_ch = [P.tile([128, 4, D], BF16, tag=f"xch{c}", name=f"xch{c}")
                for c in range(4)]

        def x_dma(eng, c):
            eng.dma_start(
                out=x_ch[c][:],
                in_=bass.AP(tensor=x_d.ap().tensor, offset=c * 512 * D,
                            ap=[[4 * D, 128], [D, 4], [1, D]]))

        def xt_dma(eng, fb, c):
            eng.dma_start(
                out=xt_ch[fb][c][:],
                in_=xt_d[fb * 128:(fb + 1) * 128, c * 512:(c + 1) * 512])

        def expa_dma(eng, q):
            eng.dma_start(
                out=expa_q[q][:],
                in_=bass.AP(tensor=expa_d.ap().tensor, offset=q * 512 * NQ,
                            ap=[[NQ, 128], [128 * NQ, 4], [1, NQ]]))

        wqkv_sb = [C.tile([128, 3 * D], BF16, tag=f"wq{kb}", name=f"wq{kb}")
                   for kb in range(4)]
        woutP = [C.tile([128, D], BF16, tag=f"woutP{hp}", name=f"woutP{hp}")
                 for hp in range(4)]
        grow_sb = C.tile([1, 3 * D], BF16, tag="grow")

        # scalar: wqkv kb2/kb3 + xt fb0/fb1 chunks 0-1 (only ~1.1 MB; the
        # sqrts behind these 4 issues still run on time).
        for kb in range(2, 4):
            nc.scalar.dma_start(out=wqkv_sb[kb][:],
                                in_=wqkv_d[kb * 128:(kb + 1) * 128, :])
        for c in range(2):
            xt_dma(nc.scalar, 0, c)
            xt_dma(nc.scalar, 1, c)
        # sync: all of x (4KB descriptors, fast), wqkv kb0/kb1, xt fb0/fb1
        # chunks 2-3, then parked expA q1-3; den/y traffic comes later.
        for c in range(4):
            x_dma(nc.sync, c)
        for kb in range(2):
            nc.sync.dma_start(out=wqkv_sb[kb][:],
                              in_=wqkv_d[kb * 128:(kb + 1) * 128, :])
        for c in range(2, 4):
            xt_dma(nc.sync, 0, c)
            xt_dma(nc.sync, 1, c)
        for q in range(1, 4):
            expa_dma(nc.sync, q)
        # gpsimd: xt fb2/fb3 chunks 0-1, grow; then the mu/rs roundtrips
        # (low latency: nothing bulky ahead), expA q0, xt fb2/fb3 rest.
        for c in range(2):
            xt_dma(nc.gpsimd, 2, c)
            xt_dma(nc.gpsimd, 3, c)
        nc.gpsimd.dma_start(out=grow_sb[:], in_=grow_d[0:1, :])

        if has_c:
            # cqkv columns are host-reordered the same way as wqkv
            cq_all = C.tile([128, 12], F32, tag="cq")
            nc.gpsimd.dma_start(
                out=cq_all[:],
                in_=bass.AP(tensor=cqkv_d.ap().tensor, offset=0,
                            ap=[[1, 128], [128, 12]]))
            cv_bc = C.tile([128, D], F32, tag="cv")
            nc.gpsimd.dma_start(
                out=cv_bc[:],
                in_=bass.AP(tensor=cqkv_d.ap().tensor, offset=2 * D,
                            ap=[[0, 128], [1, D]]))
        if has_b:
            bout_bc = C.tile([128, D], F32, tag="bout")
            nc.gpsimd.dma_start(
                out=bout_bc[:],
                in_=bass.AP(tensor=bout_d.ap().tensor, offset=0,
                            ap=[[0, 128], [1, D]]))

        def late_gpsimd_consts():
            for hp in range(4):
                nc.gpsimd.dma_start(out=woutP[hp][:],
                                    in_=wout_d[hp * 128:(hp + 1) * 128, :])

        ones1 = C.tile([1, 128], BF16, tag="ones1")
        nc.vector.memset(ones1[:], 1.0)
        eps_t = C.tile([128, 1], F32, tag="eps")
        nc.vector.memset(eps_t[:], 1e-5)

        # mu/rs roundtrip per 4-tile chunk: out on sync (f32, token order
        # mu_d[512c + 4p + t] <- mu_cc[c][p, t]), back on gpsimd (cast to
        # bf16 rows) + an f32 per-tile column view for the V evacuation.
        # All must be EMITTED after the stats that write mu_cc/rs_cc
        # (program order is logical order in Tile); backs are emitted
        # separately so they don't head-block the gpsimd queue.
        def rt_out(c, eng=None):
            eng = eng or nc.gpsimd
            sl_s = [[4, 128], [1, 4]]
            eng.dma_start(
                out=bass.AP(tensor=mu_d.ap().tensor, offset=512 * c, ap=sl_s),
                in_=mu_cc[c][:])
            eng.dma_start(
                out=bass.AP(tensor=rs_d.ap().tensor, offset=512 * c, ap=sl_s),
                in_=rs_cc[c][:])

        def rt_back(c, eng=None):
            eng = eng or nc.gpsimd
            eng.dma_start(out=murow_c[c][:],
                          in_=mu_d[0:1, 512 * c:512 * (c + 1)])
            eng.dma_start(out=rsrow_c[c][:],
                          in_=rs_d[0:1, 512 * c:512 * (c + 1)])
            nc.gpsimd.dma_start(
                out=rs_cols[c][:],
                in_=bass.AP(tensor=rs_d.ap().tensor, offset=512 * c,
                            ap=[[1, 128], [128, 4]]))

        # ---- LayerNorm stats (DVE + a gpsimd rsqrt; the ACT engine and
        # its DMA-ring-backpressured queue stay out of the critical chain) --
        # stats slot (c, s) covers tokens {512c + 4p + s : p in 0..127}
        def ln_stats(t):
            c, s = t // 4, t % 4
            st = W.tile([128, 6], F32, tag="st")
            nc.vector.bn_stats(out=st[:], in_=x_ch[c][:, s, :])
            mv = W.tile([128, 2], F32, tag="mv", bufs=4, name=f"mv{t}")
            nc.vector.bn_aggr(out=mv[:], in_=st[:])
            nc.vector.tensor_copy(out=mu_cc[c][:, s:s + 1], in_=mv[:, 0:1])
            rsq = W.tile([128, 1], F32, tag="rsq", bufs=4, name=f"rsq{t}")
            nc.scalar.activation(out=rsq[:], in_=mv[:, 1:2], func=AF.Sqrt,
                                 bias=eps_t[:])
            nc.vector.reciprocal(out=rs_cc[c][:, s:s + 1], in_=rsq[:])

        # ---- B-work closures (PE production of rstdT / K / Q / V) ----------
        with tc.tile_pool(name="psB", bufs=1, space="PSUM") as psB, \
             tc.tile_pool(name="psC", bufs=1, space="PSUM") as psC:

            def wkq(kb, hp, is_q):
                off = 256 * hp + (128 if is_q else 0)
                return wqkv_sb[kb][:, off:off + 128]

            def rstdT_build(c, on_act=False):
                def f():
                    bp = psB.tile([128, 512], F32, tag="bps", bufs=2)
                    nc.tensor.matmul(bp[:], ones1[:], rsrow_c[c][:],
                                     start=True, stop=True)
                    if on_act:
                        nc.scalar.copy(
                            out=rstdT[:, c * 512:(c + 1) * 512], in_=bp[:])
                    else:
                        nc.vector.tensor_copy(
                            out=rstdT[:, c * 512:(c + 1) * 512], in_=bp[:])
                return [f]

            def k_chunk(hp, c):
                """KTp[hp][:, c*512:(c+1)*512] (both heads' 128 rows)."""
                def mms():
                    kp = psB.tile([128, 512], F32, tag="bps", bufs=2)
                    for kb in range(4):
                        nc.tensor.matmul(
                            kp[:], wkq(kb, hp, False), xt_ch[kb][c][:],
                            start=(kb == 0), stop=False)
                    nc.tensor.matmul(
                        kp[:], grow_sb[0:1, 256 * hp:256 * hp + 128],
                        murow_c[c][:], start=False, stop=True)
                    sl = slice(c * 512, (c + 1) * 512)
                    nc.vector.tensor_mul(out=KTp[hp][:, sl], in0=kp[:],
                                         in1=rstdT[:, sl])
                    if has_c:
                        nc.vector.tensor_scalar_add(
                            out=KTp[hp][:, sl], in0=KTp[hp][:, sl],
                            scalar1=cq_all[:, 2 * hp:2 * hp + 1])
                return [mms]

            def q_chunk(hp, ic):
                """QTp[hp][:, ic*512:(ic+1)*512] (queries = tokens ic-half)."""
                def mms():
                    qp = psB.tile([128, 512], F32, tag="bps", bufs=2)
                    for kb in range(4):
                        nc.tensor.matmul(
                            qp[:], wkq(kb, hp, True), xt_ch[kb][ic][:],
                            start=(kb == 0), stop=False)
                    nc.tensor.matmul(
                        qp[:], grow_sb[0:1, 256 * hp + 128:256 * (hp + 1)],
                        murow_c[ic][:], start=False, stop=True)
                    sl = slice(ic * 512, (ic + 1) * 512)
                    nc.vector.tensor_mul(out=QTp[hp][:, sl], in0=qp[:],
                                         in1=rstdT[:, sl])
                    if has_c:
                        nc.vector.tensor_scalar_add(
                            out=QTp[hp][:, sl], in0=QTp[hp][:, sl],
                            scalar1=cq_all[:, 2 * hp + 1:2 * hp + 2])
                return [mms]

            def v_hp(t, hp):
                """Vau[t][:, 2hp:2hp+2, 0:64] = rstd * (x @ Wv_hp - mu (x) g);
                pass (hp, ic) only needs its own pair's V columns, so V
                production spreads evenly across the passes (128-col mms)."""
                def mms():
                    c, s = t // 4, t % 4
                    vp = psB.tile([128, 512], F32, tag="bps", bufs=2)
                    vsl = slice(1024 + 128 * hp, 1024 + 128 * (hp + 1))
                    for kb in range(4):
                        nc.tensor.matmul(
                            vp[:, 0:128],
                            xt_ch[kb][c][:, 128 * s:128 * (s + 1)],
                            wqkv_sb[kb][:, vsl],
                            start=(kb == 0), stop=False)
                    nc.tensor.matmul(
                        vp[:, 0:128], murow_c[c][0:1, 128 * s:128 * (s + 1)],
                        grow_sb[0:1, 2 * D + 128 * hp:2 * D + 128 * (hp + 1)],
                        start=False, stop=True)
                    if hp == 0:
                        nc.vector.memset(Vau[t][:, :, 64:65], 1.0)
                    nc.vector.tensor_scalar_mul(
                        out=Vau[t][:, 2 * hp:2 * hp + 2, 0:64],
                        in0=vp[:, 0:128], scalar1=rs_cols[c][:, s:s + 1])
                    if has_c:
                        nc.vector.tensor_add(
                            out=Vau[t][:, 2 * hp:2 * hp + 2, 0:64],
                            in0=Vau[t][:, 2 * hp:2 * hp + 2, 0:64],
                            in1=cv_bc[:, 128 * hp:128 * (hp + 1)])
                return [mms]

            # ---- denominator pipeline (unchanged from v1) ------------------
            def den_pieces(hp):
                dal = W.tile([128, 2 * NQ // 128], BF16, tag="dall", bufs=2,
                             name=f"dal{hp}")
                dbs = [None, None]

                def p0():
                    nc.scalar.dma_start(
                        out=dal[:],
                        in_=bass.AP(tensor=dsb_d.ap().tensor, offset=2 * hp * NQ,
                                    ap=[[2 * NQ // 128, 128], [1, 2 * NQ // 128]]))

                def p1():
                    nc.vector.tensor_scalar_add(out=dal[:], in0=dal[:],
                                                scalar1=1e-20)
                    with nc.allow_low_precision(reason="bf16 softmax denominators"):
                        nc.vector.reciprocal(out=dal[:], in_=dal[:])
                    nc.scalar.dma_start(
                        out=bass.AP(tensor=dsi_d.ap().tensor, offset=2 * hp * NQ,
                                    ap=[[2 * NQ // 128, 128], [1, 2 * NQ // 128]]),
                        in_=dal[:])

                def load_bc(e):
                    def f():
                        h = 2 * hp + e
                        den_bc = W.tile([64, NQ], BF16, tag="denb", bufs=2,
                                        name=f"denb{h}")
                        dbs[e] = den_bc
                        nc.scalar.dma_start(
                            out=den_bc[:],
                            in_=bass.AP(tensor=dsi_d.ap().tensor, offset=h * NQ,
                                        ap=[[0, 64], [1, NQ]]))
                    return f

                def mul_chunk(e, half):
                    def f():
                        h = 2 * hp + e
                        sl = slice(half * 512, (half + 1) * 512)
                        if e == 0:
                            nc.vector.tensor_mul(out=pairT[hp][0:64, sl],
                                                 in0=numT[h][0:64, sl],
                                                 in1=dbs[e][:, sl])
                        else:
                            nc.vector.tensor_mul(out=numT[h][0:64, sl],
                                                 in0=numT[h][0:64, sl],
                                                 in1=dbs[e][:, sl])
                    return f

                def stitch():
                    nc.scalar.dma_start(out=pairT[hp][64:128, :],
                                      in_=numT[2 * hp + 1][0:64, :])

                return [p0, None, None, None, p1, None, load_bc(0),
                        load_bc(1), None, None, mul_chunk(0, 0),
                        mul_chunk(0, 1), mul_chunk(1, 0), mul_chunk(1, 1),
                        stitch]

            # ---- prologue: stats waves + roundtrips + parked bulk DMA ------
            for t in range(4):
                ln_stats(t)
            rt_out(0)
            rt_back(0)
            for t in range(4, 8):
                ln_stats(t)
            rt_out(1)
            rt_back(1)
            expa_dma(nc.gpsimd, 0)
            xt_dma(nc.gpsimd, 2, 2)
            xt_dma(nc.gpsimd, 3, 2)
            xt_dma(nc.gpsimd, 2, 3)
            xt_dma(nc.gpsimd, 3, 3)
            for t in range(8, 12):
                ln_stats(t)
            rt_out(2)
            rt_back(2)

            # PE warm-up (HAM) while waiting for the mu/rs roundtrip, then
            # everything iter0 needs: rstdT c0, K[0,0], Q[0,0].
            for wu in range(30):
                dmy = psB.tile([128, 512], F32, tag="bps", bufs=2,
                               name=f"wu{wu}")
                nc.tensor.matmul(dmy[:], xt_ch[2][0][:, 0:128], xt_ch[2][0][:],
                                 start=True, stop=True)
            rstdT_build(0, on_act=True)[0]()
            k_chunk(0, 0)[0]()
            q_chunk(0, 0)[0]()

            # Remaining B-work, popped 2/iter inside C. Order respects
            # both data deadlines and global program-order hazards
            # (producers must be emitted before their consumers).
            workq = [lambda t=t: ln_stats(t) for t in range(12, NT)]
            workq += [lambda: (rt_out(3, nc.gpsimd), rt_back(3),
                               late_gpsimd_consts())]
            workq += rstdT_build(1)
            workq += k_chunk(0, 1)
            workq += v_hp(0, 0)
            workq += v_hp(1, 0)
            workq += rstdT_build(2)
            workq += k_chunk(0, 2)
            workq += v_hp(2, 0)
            workq += v_hp(3, 0)
            workq += rstdT_build(3)
            workq += k_chunk(0, 3)
            workq += v_hp(4, 0)
            workq += v_hp(5, 0)
            workq += v_hp(6, 0)
            workq += q_chunk(0, 1)
            for t in range(7, NT):
                workq += v_hp(t, 0)
            # production for later passes: pass p uses K[p//2], Q[p//2, p%2]
            # and V columns of pair p//2; all of it drains a pass ahead.
            for hp in range(1, 4):
                for c in range(4):
                    workq += k_chunk(hp, c)
                workq += q_chunk(hp, 0)
                workq += q_chunk(hp, 1)
                for t in range(NT):
                    workq += v_hp(t, hp)

            # ---- Phase C: 8 passes x 16 key tiles --------------------------
            avq = []   # (closure) AV work trailing the exp stream

            def av_mms(p, jt, pbt, av0, av1):
                def f():
                    hp = p // 2
                    h0, h1 = 2 * hp, 2 * hp + 1
                    nc.tensor.matmul(av0[:], Vau[jt][:, h0, 0:65],
                                     pbt[:, 0:512],
                                     start=(jt == 0), stop=(jt == NT - 1))
                    nc.tensor.matmul(av1[:], Vau[jt][:, h1, 0:65],
                                     pbt[:, 512:1024],
                                     start=(jt == 0), stop=(jt == NT - 1))
                return f

            for p in range(8):
                hp, ic = p // 2, p % 2
                av0 = psC.tile([65, 512], F32, tag="av0", bufs=1,
                               name=f"av0_{p}")
                av1 = psC.tile([65, 512], F32, tag="av1", bufs=1,
                               name=f"av1_{p}")
                i5 = ic * 512
                for jt in range(NT):
                    sp = psC.tile([128, 1024], F32, tag="sp", bufs=2)
                    nc.tensor.matmul(
                        sp[:, 0:512],
                        KTp[hp][0:64, jt * 128:(jt + 1) * 128],
                        QTp[hp][0:64, i5:i5 + 512],
                        start=True, stop=True, tile_position=(0, 0))
                    nc.tensor.matmul(
                        sp[:, 512:1024],
                        KTp[hp][64:128, jt * 128:(jt + 1) * 128],
                        QTp[hp][64:128, i5:i5 + 512],
                        start=True, stop=True, tile_position=(64, 0))
                    eb = W.tile([128, 1024], BF16, tag="eb", bufs=3)
                    nc.scalar.activation(out=eb[:], in_=sp[:], func=AF.Exp)
                    pbt = W.tile([128, 1024], BF16, tag="pb", bufs=LAG + 2)
                    ea = expa_q[jt // 4][:, jt % 4, i5:i5 + 512]
                    nc.vector.tensor_mul(out=pbt[:, 0:512], in0=eb[:, 0:512],
                                         in1=ea)
                    nc.vector.tensor_mul(out=pbt[:, 512:1024],
                                         in0=eb[:, 512:1024], in1=ea)
                    avq.append(av_mms(p, jt, pbt, av0, av1))
                    # drain trailing B-work (2/iter keeps producers ahead
                    # of their PE-queue consumers), then lagged AV work
                    # (also at most 2/iter so den roundtrips stay spread).
                    for _ in range(2):
                        if workq:
                            workq.pop(0)()
                    for _ in range(2):
                        if len(avq) > LAG:
                            fn = avq.pop(0)
                            if fn is not None:
                                fn()
                # end of pass: trailing AV for this pass still in avq; queue
                # the evacuation work behind them.
                def pass_tail(p=p, av0=av0, av1=av1):
                    hp, ic = p // 2, p % 2
                    h0, h1 = 2 * hp, 2 * hp + 1
                    sl = slice(ic * 512, (ic + 1) * 512)
                    def f():
                        nc.vector.tensor_copy(out=numT[h0][:, sl], in_=av0[:])
                        nc.vector.tensor_copy(out=numT[h1][:, sl], in_=av1[:])
                        if ic == 1:
                            for e in range(2):
                                h = 2 * hp + e
                                nc.scalar.dma_start(out=dsb_d[h, :],
                                                    in_=numT[h][64:65, :])
                    return f
                avq.append(pass_tail())
                if p % 2 == 1:
                    avq.extend(den_pieces(p // 2))

            # flush remaining trailing work, keeping the PE busy through the
            # last denominator roundtrip (dummy matmuls bridge HAM warmth)
            tail = [fn for fn in avq if fn is not None]
            for i, fn in enumerate(tail):
                fn()
                if i % 2 == 1:
                    dmy = psB.tile([128, 512], F32, tag="bps", bufs=2,
                                   name=f"dmy{i}")
                    nc.tensor.matmul(dmy[:], wqkv_sb[0][:, 0:128],
                                     xt_ch[0][0][:], start=True, stop=True)

        # ---- Phase D: output projection (head pairs, K=128) ----------------
        with tc.tile_pool(name="psD", bufs=1, space="PSUM") as psD:
            yps = [psD.tile([128, 512], F32, tag=f"yp{isl}", name=f"yp{isl}")
                   for isl in range(8)]
            for hp in range(4):
                for isl in range(8):
                    nc.tensor.matmul(yps[isl][:],
                                     pairT[hp][:, isl * 128:(isl + 1) * 128],
                                     woutP[hp][:],
                                     start=(hp == 0), stop=(hp == 3))
            for isl in range(8):
                ysb = W.tile([128, 512], F32, tag="ysb", bufs=2)
                if has_b:
                    nc.vector.tensor_add(out=ysb[:], in0=yps[isl][:],
                                         in1=bout_bc[:])
                elif isl % 2 == 0:
                    nc.vector.tensor_copy(out=ysb[:], in_=yps[isl][:])
                else:
                    nc.scalar.copy(out=ysb[:], in_=yps[isl][:])
                nc.sync.dma_start(out=y_d[isl * 128:(isl + 1) * 128, :],
                                  in_=ysb[:])
    if split_waits:
        _split_waits(nc)
    return nc


_NC_CACHE = {}


def _get_nc(has_c, has_b):
    key = (has_c, has_b)
    if key not in _NC_CACHE:
        _NC_CACHE[key] = build(has_c, has_b)
    return _NC_CACHE[key]


LAST_EXEC_TIME_NS = None


def kernel(x, gamma, beta, Wqkv, Wout, bout, rel_table, temporal_mask,
           trace=True):
    global LAST_EXEC_TIME_NS
    x = np.asarray(x, np.float32)
    gamma = np.asarray(gamma, np.float32)
    beta = np.asarray(beta, np.float32)
    Wqkv = np.asarray(Wqkv, np.float32)
    Wout = np.asarray(Wout, np.float32)
    bout = np.asarray(bout, np.float32)
    rel_table = np.asarray(rel_table, np.float32)
    temporal_mask = np.asarray(temporal_mask)

    scale = DH ** -0.5
    w_eff = (Wqkv * gamma[:, None]).copy()
    w_eff[:, :D] *= scale
    cqkv = (beta @ Wqkv).astype(np.float32)
    cqkv[:D] *= scale
    # column reorder: [K0 Q0 K1 Q1 K2 Q2 K3 Q3 V] so hp0's K/Q land first
    perm = []
    for hp in range(H // 2):
        perm += list(range(D + hp * 128, D + (hp + 1) * 128))
        perm += list(range(hp * 128, (hp + 1) * 128))
    perm += list(range(2 * D, 3 * D))
    w_eff = np.ascontiguousarray(w_eff[:, perm])
    cqkv = np.ascontiguousarray(cqkv[perm])
    wqkv_bf = w_eff.astype(ml_dtypes.bfloat16)
    # rank-1 mean correction: -g rows (so the matmul accumulates -mu*g)
    grow_bf = (-w_eff.sum(axis=0)).reshape(1, -1).astype(ml_dtypes.bfloat16)
    wout_bf = Wout.astype(ml_dtypes.bfloat16)
    mask01 = (temporal_mask != 0)

    idx = np.arange(N)
    # expbias[i, j] = exp(rel_table[clip(i - j)]) with i=query, j=key
    expbias = np.exp(rel_table[
        np.clip(idx[:, None] - idx[None, :], -(MAXREL - 1), MAXREL - 1)
        + MAXREL - 1]).astype(np.float32)

    keyperm_half = [
        np.concatenate([np.arange(i0, i0 + NQ),
                        np.arange(NQ - i0, NQ - i0 + NQ)])
        for i0 in (0, NQ)
    ]
    # expA[j_perm, i_local] = exp(bias(query i, key j)) * mask(query i, key j)
    expa_half = []
    for half in range(2):
        kp = keyperm_half[half]
        qs = np.arange(half * NQ, (half + 1) * NQ)
        a = (expbias[np.ix_(qs, kp)] * mask01[np.ix_(qs, kp)]).T
        expa_half.append(np.ascontiguousarray(a).astype(ml_dtypes.bfloat16))

    in_maps = []
    for c in range(NCORES):
        b, half = c // 2, c % 2
        xp = np.ascontiguousarray(x[b][keyperm_half[half]])
        xtp = np.ascontiguousarray(xp.T).astype(ml_dtypes.bfloat16)
        in_maps.append({
            "x": xp.astype(ml_dtypes.bfloat16),
            "xt": xtp,
            "wqkv": wqkv_bf,
            "grow": grow_bf,
            "cqkv": cqkv,
            "wout": wout_bf,
            "bout": bout,
            "expa": expa_half[half],
        })

    nc = _get_nc(bool(np.any(cqkv != 0.0)), bool(np.any(bout != 0.0)))
    res = run_bass_kernel_spmd(nc, in_maps, core_ids=list(range(NCORES)),
                               trace=trace)
    LAST_EXEC_TIME_NS = res.exec_time_ns

    out = np.empty((B, N, D), np.float32)
    for c in range(NCORES):
        b, half = c // 2, c % 2
        out[b, half * NQ:(half + 1) * NQ] = res.results[c]["y"]
    return out


# revision 59
# speedup vs baseline: 1.0462x; 1.0242x over previous
"""Trainium2 Bass kernel for nn_Attention_8323646620215.

LayerNorm -> QKV -> scores(+rel-bias+mask) -> softmax -> attn@V -> out proj.

Sharding: 8 cores = (batch b in 0..3) x (query-half in 0..1). Each core
computes the full K/V for its batch and attention for its 1024 query rows;
no cross-core communication.

v2 design: one ACT-exp-bound pipeline. The softmax exp (128 x [128,1024]
f32 PSUM reads on the scalar engine) is the hard floor (~128us); everything
else is scheduled into the other engines' slack under it:

  - No on-chip transposes: the host also sends xT = x.T (bf16). Q/K/V are
    computed from RAW xT (un-normalized); LayerNorm enters algebraically:
      xn = (x - mu) * rstd  (gamma/beta folded into weights host-side)
      K_hat[out, tok] = rstd[tok] * (W.T @ xT - g (x) mu)[out, tok]
    The rank-1 mean term (g = colsum(W)) is one extra contraction-1 matmul
    accumulated into the same PSUM group; the per-token rstd scale is
    applied at PSUM evacuation (DVE tensor_tensor with a broadcast rstd_T
    tile built on-chip by a rank-1 PE matmul).
  - mu/rstd come from DVE bn_stats on the f32 x, shipped through a tiny
    DRAM roundtrip (gpsimd cast-DMA back as bf16 rows).
  - expA = exp(clip rel bias) * mask is precomputed host-side ([N, NQ]
    bf16, same bytes as the old mask DMA) - no Toeplitz build on-chip.
  - Phase C runs 8 passes (head-pair x query-half) x 16 key tiles:
    scores (2 row-packed 64-contraction matmuls) -> exp -> pb = eb*expA
    (stride-0 repeated read) -> attn@V (65-row stationary, ones row gives
    the softmax denominator). AV matmuls run LAG iterations behind the exp
    stream so V/K/Q production for later passes interleaves into PE slack.
  - Denominators: DRAM roundtrip reshape -> reciprocal -> broadcast read
    (as in v1), interleaved into the following pass.
"""
import sys
import types
import numpy as np

sys.path.insert(0, "/opt/trn_rl_repo")

# ---- environment fixes (axon agent container) -------------------------------
if "antenv.axon_hooks" not in sys.modules:
    _m = types.ModuleType("antenv.axon_hooks")
    _m._hook = None
    _m.set_axon_ntff_profile_hook = lambda h: setattr(_m, "_hook", h)
    _m.get_axon_ntff_profile_hook = lambda: _m._hook
    sys.modules["antenv.axon_hooks"] = _m
    try:
        from trn_agent_boot.trn_boot import _ntff_profile_via_ctypes
        _m._hook = _ntff_profile_via_ctypes("/opt/axon/libaxon_pjrt.so")
    except Exception:
        pass

import ml_dtypes  # noqa: E402
from concourse import bass, mybir, tile  # noqa: E402
from concourse.bass_utils import run_bass_kernel_spmd  # noqa: E402

F32 = mybir.dt.float32
BF16 = mybir.dt.bfloat16
AF = mybir.ActivationFunctionType
OP = mybir.AluOpType

B, N, D, H, DH, MAXREL = 4, 2048, 512, 8, 64, 200
NQ = N // 2          # queries per core
NT = N // 128        # 16 token tiles
NCORES = 8
LAG = 6              # AV matmuls trail the exp stream by this many iters

# This container's walrus rejects instructions with more than one sem wait.
# Splitting is sound: a same-engine NoOp right before the instruction
# enforces the wait at the same program point (sequencers run in order).


def _split_waits(nc, maxw=1):
    n_split = 0
    for f in nc.m.functions:
        for blk in f.blocks:
            bb = blk.bb if hasattr(blk, "bb") else blk
            insts = list(bb.instructions)
            out = []
            changed = False
            for inst in insts:
                si = inst.sync_info
                waits = list(si.on_wait) if si and si.on_wait else []
                if len(waits) > maxw:
                    extra = waits[:-maxw]
                    chunks = [extra[j:j + maxw] for j in range(0, len(extra), maxw)]
                    for i, chunk in enumerate(chunks):
                        nop = mybir.InstNoOp(name=f"{inst.name}-ws{i}", ins=[], outs=[])
                        nop.engine = inst.engine
                        nop.sync_info = mybir.SyncInfo(on_wait=chunk, on_update=[])
                        out.append(nop)
                    si.on_wait = waits[-maxw:]
                    changed = True
                    n_split += 1
                out.append(inst)
            if changed:
                bb.instructions = out
    return n_split


def build(has_c=False, has_b=False, split_waits=True):
    nc = bass.Bass("TRN2", target_bir_lowering=False, debug=False,
                   num_devices=NCORES)
    x_d = nc.dram_tensor("x", [N, D], BF16, kind="ExternalInput")
    xt_d = nc.dram_tensor("xt", [D, N], BF16, kind="ExternalInput")
    wqkv_d = nc.dram_tensor("wqkv", [D, 3 * D], BF16, kind="ExternalInput")
    grow_d = nc.dram_tensor("grow", [1, 3 * D], BF16, kind="ExternalInput")
    wout_d = nc.dram_tensor("wout", [D, D], BF16, kind="ExternalInput")
    bout_d = nc.dram_tensor("bout", [D], F32, kind="ExternalInput")
    cqkv_d = nc.dram_tensor("cqkv", [3 * D], F32, kind="ExternalInput")
    expa_d = nc.dram_tensor("expa", [N, NQ], BF16, kind="ExternalInput")
    mu_d = nc.dram_tensor("mu_scratch", [1, N], F32)
    rs_d = nc.dram_tensor("rs_scratch", [1, N], F32)
    dsb_d = nc.dram_tensor("den_scratch", [H, NQ], BF16)
    dsi_d = nc.dram_tensor("invden_scratch", [H, NQ], BF16)
    y_d = nc.dram_tensor("y", [NQ, D], F32, kind="ExternalOutput")

    with tile.TileContext(nc) as tc, \
         tc.tile_pool(name="const", bufs=1) as C, \
         tc.tile_pool(name="pers", bufs=1) as P, \
         tc.tile_pool(name="work", bufs=3) as W:

        # ---- persistent tiles ----------------------------------------------
        # Every DMA-written tile is written by exactly ONE dma_start (Tile's
        # write-hazard tracking is coarse; multi-DMA tiles make any consumer
        # wait for the LAST write to the tile).
        xt_ch = [[P.tile([128, 512], BF16, tag=f"xt{fb}_{c}",
                         name=f"xt{fb}_{c}") for c in range(4)]
                 for fb in range(4)]
        KTp = [P.tile([128, N], BF16, tag=f"KT{hp}", name=f"KT{hp}") for hp in range(4)]
        QTp = [P.tile([128, NQ], BF16, tag=f"QT{hp}", name=f"QT{hp}") for hp in range(4)]
        Vau = [P.tile([128, H, 66], BF16, tag=f"V{t}", name=f"Vau{t}") for t in range(NT)]
        expa_q = [P.tile([128, 4, NQ], BF16, tag=f"eA{q}", name=f"eA{q}")
                  for q in range(4)]
        numT = [P.tile([65, NQ], BF16, tag=f"nT{h}", name=f"nT{h}") for h in range(H)]
        pairT = [P.tile([128, NQ], BF16, tag=f"pT{hp}", name=f"pT{hp}") for hp in range(4)]
        rstdT = P.tile([128, N], BF16, tag="rstdT", name="rstdT")
        mu_cc = [P.tile([128, 4], F32, tag=f"mu{c}", name=f"mu{c}")
                 for c in range(4)]
        rs_cc = [P.tile([128, 4], F32, tag=f"rs{c}", name=f"rs{c}")
                 for c in range(4)]
        murow_c = [P.tile([1, 512], BF16, tag=f"mur{c}", name=f"mur{c}")
                   for c in range(4)]
        rsrow_c = [P.tile([1, 512], BF16, tag=f"rsr{c}", name=f"rsr{c}")
                   for c in range(4)]
        rs_cols = [P.tile([128, 4], F32, tag=f"rsc{c}", name=f"rsc{c}")
                   for c in range(4)]

        # ---- DMA issue plan ------------------------------------------------
        # Each dma_start costs ~600ns on its issuing engine and queue
        # bandwidth depends on descriptor size (contiguous run length), so:
        # x is partition-remapped so each partition holds 4 consecutive
        # DRAM rows (4KB descriptors), queues are load-balanced and ordered
        # by consumer deadline, and wqkv columns are host-reordered to
        # [K0 Q0 K1 Q1 K2 Q2 K3 Q3 V].
        # x_ch[c][p, t, f] = x[512c + 4p + t, f]  (token = 512c + 4p + t)
        x_ch = [P.tile([128, 4, D], BF16, tag=f"xch{c}", name=f"xch{c}")
                for c in range(4)]

        def x_dma(eng, c):
            eng.dma_start(
                out=x_ch[c][:],
                in_=bass.AP(tensor=x_d.ap().tensor, offset=c * 512 * D,
                            ap=[[4 * D, 128], [D, 4], [1, D]]))

        def xt_dma(eng, fb, c):
            eng.dma_start(
                out=xt_ch[fb][c][:],
                in_=xt_d[fb * 128:(fb + 1) * 128, c * 512:(c + 1) * 512])

        def expa_dma(eng, q):
            eng.dma_start(
                out=expa_q[q][:],
                in_=bass.AP(tensor=expa_d.ap().tensor, offset=q * 512 * NQ,
                            ap=[[NQ, 128], [128 * NQ, 4], [1, NQ]]))

        wqkv_sb = [C.tile([128, 3 * D], BF16, tag=f"wq{kb}", name=f"wq{kb}")
                   for kb in range(4)]
        woutP = [C.tile([128, D], BF16, tag=f"woutP{hp}", name=f"woutP{hp}")
                 for hp in range(4)]
        grow_sb = C.tile([1, 3 * D], BF16, tag="grow")

        # scalar: wqkv kb2/kb3 + xt fb0/fb1 chunks 0-1 (only ~1.1 MB; the
        # sqrts behind these 4 issues still run on time).
        for kb in range(2, 4):
            nc.scalar.dma_start(out=wqkv_sb[kb][:],
                                in_=wqkv_d[kb * 128:(kb + 1) * 128, :])
        for c in range(2):
            xt_dma(nc.scalar, 0, c)
            xt_dma(nc.scalar, 1, c)
        # sync: all of x (4KB descriptors, fast), wqkv kb0/kb1, xt fb0/fb1
        # chunks 2-3, then parked expA q1-3; den/y traffic comes later.
        for c in range(4):
            x_dma(nc.sync, c)
        for kb in range(2):
            nc.sync.dma_start(out=wqkv_sb[kb][:],
                              in_=wqkv_d[kb * 128:(kb + 1) * 128, :])
        for c in range(2, 4):
            xt_dma(nc.sync, 0, c)
            xt_dma(nc.sync, 1, c)
        for q in range(1, 4):
            expa_dma(nc.sync, q)
        # gpsimd: xt fb2/fb3 chunks 0-1, grow; then the mu/rs roundtrips
        # (low latency: nothing bulky ahead), expA q0, xt fb2/fb3 rest.
        for c in range(2):
            xt_dma(nc.gpsimd, 2, c)
            xt_dma(nc.gpsimd, 3, c)
        nc.gpsimd.dma_start(out=grow_sb[:], in_=grow_d[0:1, :])

        if has_c:
            # cqkv columns are host-reordered the same way as wqkv
            cq_all = C.tile([128, 12], F32, tag="cq")
            nc.gpsimd.dma_start(
                out=cq_all[:],
                in_=bass.AP(tensor=cqkv_d.ap().tensor, offset=0,
                            ap=[[1, 128], [128, 12]]))
            cv_bc = C.tile([128, D], F32, tag="cv")
            nc.gpsimd.dma_start(
                out=cv_bc[:],
                in_=bass.AP(tensor=cqkv_d.ap().tensor, offset=2 * D,
                            ap=[[0, 128], [1, D]]))
        if has_b:
            bout_bc = C.tile([128, D], F32, tag="bout")
            nc.gpsimd.dma_start(
                out=bout_bc[:],
                in_=bass.AP(tensor=bout_d.ap().tensor, offset=0,
                            ap=[[0, 128], [1, D]]))

        def late_gpsimd_consts():
            for hp in range(4):
                nc.gpsimd.dma_start(out=woutP[hp][:],
                                    in_=wout_d[hp * 128:(hp + 1) * 128, :])

        ones1 = C.tile([1, 128], BF16, tag="ones1")
        nc.vector.memset(ones1[:], 1.0)
        eps_t = C.tile([128, 1], F32, tag="eps")
        nc.vector.memset(eps_t[:], 1e-5)

        # mu/rs roundtrip per 4-tile chunk: out on sync (f32, token order
        # mu_d[512c + 4p + t] <- mu_cc[c][p, t]), back on gpsimd (cast to
        # bf16 rows) + an f32 per-tile column view for the V evacuation.
        # All must be EMITTED after the stats that write mu_cc/rs_cc
        # (program order is logical order in Tile); backs are emitted
        # separately so they don't head-block the gpsimd queue.
        def rt_out(c, eng=None):
            eng = eng or nc.gpsimd
            sl_s = [[4, 128], [1, 4]]
            eng.dma_start(
                out=bass.AP(tensor=mu_d.ap().tensor, offset=512 * c, ap=sl_s),
                in_=mu_cc[c][:])
            eng.dma_start(
                out=bass.AP(tensor=rs_d.ap().tensor, offset=512 * c, ap=sl_s),
                in_=rs_cc[c][:])

        def rt_back(c, eng=None):
            eng = eng or nc.gpsimd
            eng.dma_start(out=murow_c[c][:],
                          in_=mu_d[0:1, 512 * c:512 * (c + 1)])
            eng.dma_start(out=rsrow_c[c][:],
                          in_=rs_d[0:1, 512 * c:512 * (c + 1)])
            nc.gpsimd.dma_start(
                out=rs_cols[c][:],
                in_=bass.AP(tensor=rs_d.ap().tensor, offset=512 * c,
                            ap=[[1, 128], [128, 4]]))

        # ---- LayerNorm stats (DVE + a gpsimd rsqrt; the ACT engine and
        # its DMA-ring-backpressured queue stay out of the critical chain) --
        # stats slot (c, s) covers tokens {512c + 4p + s : p in 0..127}
        def ln_stats(t):
            c, s = t // 4, t % 4
            st = W.tile([128, 6], F32, tag="st")
            nc.vector.bn_stats(out=st[:], in_=x_ch[c][:, s, :])
            mv = W.tile([128, 2], F32, tag="mv", bufs=4, name=f"mv{t}")
            nc.vector.bn_aggr(out=mv[:], in_=st[:])
            nc.vector.tensor_copy(out=mu_cc[c][:, s:s + 1], in_=mv[:, 0:1])
            rsq = W.tile([128, 1], F32, tag="rsq", bufs=4, name=f"rsq{t}")
            nc.scalar.activation(out=rsq[:], in_=mv[:, 1:2], func=AF.Sqrt,
                                 bias=eps_t[:])
            nc.vector.reciprocal(out=rs_cc[c][:, s:s + 1], in_=rsq[:])

        # ---- B-work closures (PE production of rstdT / K / Q / V) ----------
        with tc.tile_pool(name="psB", bufs=1, space="PSUM") as psB, \
             tc.tile_pool(name="psC", bufs=1, space="PSUM") as psC:

            def wkq(kb, hp, is_q):
                off = 256 * hp + (128 if is_q else 0)
                return wqkv_sb[kb][:, off:off + 128]

            def rstdT_build(c, on_act=False):
                def f():
                    bp = psB.tile([128, 512], F32, tag="bps", bufs=2)
                    nc.tensor.matmul(bp[:], ones1[:], rsrow_c[c][:],
                                     start=True, stop=True)
                    if on_act:
                        nc.scalar.copy(
                            out=rstdT[:, c * 512:(c + 1) * 512], in_=bp[:])
                    else:
                        nc.vector.tensor_copy(
                            out=rstdT[:, c * 512:(c + 1) * 512], in_=bp[:])
                return [f]

            def k_chunk(hp, c):
                """KTp[hp][:, c*512:(c+1)*512] (both heads' 128 rows)."""
                def mms():
                    kp = psB.tile([128, 512], F32, tag="bps", bufs=2)
                    for kb in range(4):
                        nc.tensor.matmul(
                            kp[:], wkq(kb, hp, False), xt_ch[kb][c][:],
                            start=(kb == 0), stop=False)
                    nc.tensor.matmul(
                        kp[:], grow_sb[0:1, 256 * hp:256 * hp + 128],
                        murow_c[c][:], start=False, stop=True)
                    sl = slice(c * 512, (c + 1) * 512)
                    nc.vector.tensor_mul(out=KTp[hp][:, sl], in0=kp[:],
                                         in1=rstdT[:, sl])
                    if has_c:
                        nc.vector.tensor_scalar_add(
                            out=KTp[hp][:, sl], in0=KTp[hp][:, sl],
                            scalar1=cq_all[:, 2 * hp:2 * hp + 1])
                return [mms]

            def q_chunk(hp, ic):
                """QTp[hp][:, ic*512:(ic+1)*512] (queries = tokens ic-half)."""
                def mms():
                    qp = psB.tile([128, 512], F32, tag="bps", bufs=2)
                    for kb in range(4):
                        nc.tensor.matmul(
                            qp[:], wkq(kb, hp, True), xt_ch[kb][ic][:],
                            start=(kb == 0), stop=False)
                    nc.tensor.matmul(
                        qp[:], grow_sb[0:1, 256 * hp + 128:256 * (hp + 1)],
                        murow_c[ic][:], start=False, stop=True)
                    sl = slice(ic * 512, (ic + 1) * 512)
                    nc.vector.tensor_mul(out=QTp[hp][:, sl], in0=qp[:],
                                         in1=rstdT[:, sl])
                    if has_c:
                        nc.vector.tensor_scalar_add(
                            out=QTp[hp][:, sl], in0=QTp[hp][:, sl],
                            scalar1=cq_all[:, 2 * hp + 1:2 * hp + 2])
                return [mms]

            def v_hp(t, hp):
                """Vau[t][:, 2hp:2hp+2, 0:64] = rstd * (x @ Wv_hp - mu (x) g);
                pass (hp, ic) only needs its own pair's V columns, so V
                production spreads evenly across the passes (128-col mms)."""
                def mms():
                    c, s = t // 4, t % 4
                    vp = psB.tile([128, 512], F32, tag="bps", bufs=2)
                    vsl = slice(1024 + 128 * hp, 1024 + 128 * (hp + 1))
                    for kb in range(4):
                        nc.tensor.matmul(
                            vp[:, 0:128],
                            xt_ch[kb][c][:, 128 * s:128 * (s + 1)],
                            wqkv_sb[kb][:, vsl],
                            start=(kb == 0), stop=False)
                    nc.tensor.matmul(
                        vp[:, 0:128], murow_c[c][0:1, 128 * s:128 * (s + 1)],
                        grow_sb[0:1, 2 * D + 128 * hp:2 * D + 128 * (hp + 1)],
                        start=False, stop=True)
                    if hp == 0:
                        nc.vector.memset(Vau[t][:, :, 64:65], 1.0)
                    nc.vector.tensor_scalar_mul(
                        out=Vau[t][:, 2 * hp:2 * hp + 2, 0:64],
                        in0=vp[:, 0:128], scalar1=rs_cols[c][:, s:s + 1])
                    if has_c:
                        nc.vector.tensor_add(
                            out=Vau[t][:, 2 * hp:2 * hp + 2, 0:64],
                            in0=Vau[t][:, 2 * hp:2 * hp + 2, 0:64],
                            in1=cv_bc[:, 128 * hp:128 * (hp + 1)])
                return [mms]

            # ---- denominator pipeline (unchanged from v1) ------------------
            def den_pieces(hp):
                dal = W.tile([128, 2 * NQ // 128], BF16, tag="dall", bufs=2,
                             name=f"dal{hp}")
                dbs = [None, None]

                def p0():
                    nc.scalar.dma_start(
                        out=dal[:],
                        in_=bass.AP(tensor=dsb_d.ap().tensor, offset=2 * hp * NQ,
                                    ap=[[2 * NQ // 128, 128], [1, 2 * NQ // 128]]))

                def p1():
                    nc.vector.tensor_scalar_add(out=dal[:], in0=dal[:],
                                                scalar1=1e-20)
                    with nc.allow_low_precision(reason="bf16 softmax denominators"):
                        nc.vector.reciprocal(out=dal[:], in_=dal[:])
                    nc.scalar.dma_start(
                        out=bass.AP(tensor=dsi_d.ap().tensor, offset=2 * hp * NQ,
                                    ap=[[2 * NQ // 128, 128], [1, 2 * NQ // 128]]),
                        in_=dal[:])

                def load_bc(e):
                    def f():
                        h = 2 * hp + e
                        den_bc = W.tile([64, NQ], BF16, tag="denb", bufs=2,
                                        name=f"denb{h}")
                        dbs[e] = den_bc
                        nc.scalar.dma_start(
                            out=den_bc[:],
                            in_=bass.AP(tensor=dsi_d.ap().tensor, offset=h * NQ,
                                        ap=[[0, 64], [1, NQ]]))
                    return f

                def mul_chunk(e, half):
                    def f():
                        h = 2 * hp + e
                        sl = slice(half * 512, (half + 1) * 512)
                        if e == 0:
                            nc.vector.tensor_mul(out=pairT[hp][0:64, sl],
                                                 in0=numT[h][0:64, sl],
                                                 in1=dbs[e][:, sl])
                        else:
                            nc.vector.tensor_mul(out=numT[h][0:64, sl],
                                                 in0=numT[h][0:64, sl],
                                                 in1=dbs[e][:, sl])
                    return f

                def stitch():
                    nc.scalar.dma_start(out=pairT[hp][64:128, :],
                                      in_=numT[2 * hp + 1][0:64, :])

                return [p0, None, None, None, p1, None, load_bc(0),
                        load_bc(1), None, None, mul_chunk(0, 0),
                        mul_chunk(0, 1), mul_chunk(1, 0), mul_chunk(1, 1),
                        stitch]

            # ---- prologue: stats waves + roundtrips + parked bulk DMA ------
            for t in range(4):
                ln_stats(t)
            rt_out(0)
            rt_back(0)
            for t in range(4, 8):
                ln_stats(t)
            rt_out(1)
            rt_back(1)
            expa_dma(nc.gpsimd, 0)
            xt_dma(nc.gpsimd, 2, 2)
            xt_dma(nc.gpsimd, 3, 2)
            xt_dma(nc.gpsimd, 2, 3)
            xt_dma(nc.gpsimd, 3, 3)
            for t in range(8, 12):
                ln_stats(t)
            rt_out(2)
            rt_back(2)

            # PE warm-up (HAM) while waiting for the mu/rs roundtrip, then
            # everything iter0 needs: rstdT c0, K[0,0], Q[0,0].
            for wu in range(30):
                dmy = psB.tile([128, 512], F32, tag="bps", bufs=2,
                               name=f"wu{wu}")
                nc.tensor.matmul(dmy[:], xt_ch[2][0][:, 0:128], xt_ch[2][0][:],
                                 start=True, stop=True)
            rstdT_build(0, on_act=True)[0]()
            k_chunk(0, 0)[0]()
            q_chunk(0, 0)[0]()

            # Remaining B-work, popped 2/iter inside C. Order respects
            # both data deadlines and global program-order hazards
            # (producers must be emitted before their consumers).
            # Entries are (PE-weight ns, closure); the C loop drains by a
            # per-iteration PE budget so deferred production never floods
            # the PE queue ahead of the score/exp/AV stream.
            W_K, W_V, W_R = 1100, 350, 250
            workq = [(0, lambda t=t: ln_stats(t)) for t in range(12, NT)]
            workq += [(0, lambda: (rt_out(3, nc.gpsimd), rt_back(3),
                                   late_gpsimd_consts()))]
            workq += [(W_R, rstdT_build(1)[0])]
            workq += [(W_K, k_chunk(0, 1)[0])]
            workq += [(W_V, v_hp(0, 0)[0])]
            workq += [(W_V, v_hp(1, 0)[0])]
            workq += [(W_R, rstdT_build(2)[0])]
            workq += [(W_K, k_chunk(0, 2)[0])]
            workq += [(W_V, v_hp(2, 0)[0])]
            workq += [(W_V, v_hp(3, 0)[0])]
            workq += [(W_R, rstdT_build(3)[0])]
            workq += [(W_K, k_chunk(0, 3)[0])]
            workq += [(W_V, v_hp(4, 0)[0])]
            workq += [(W_V, v_hp(5, 0)[0])]
            workq += [(W_V, v_hp(6, 0)[0])]
            workq += [(W_K, q_chunk(0, 1)[0])]
            for t in range(7, NT):
                workq += [(W_V, v_hp(t, 0)[0])]
            # production for later passes: pass p uses K[p//2], Q[p//2, p%2]
            # and V columns of pair p//2; K/Q drain a pass ahead, V by the
            # consuming pass's own AV lag.
            for hp in range(1, 4):
                for c in range(4):
                    workq += [(W_K, k_chunk(hp, c)[0])]
                workq += [(W_K, q_chunk(hp, 0)[0])]
                workq += [(W_K, q_chunk(hp, 1)[0])]
                for t in range(NT):
                    workq += [(W_V, v_hp(t, hp)[0])]

            # ---- Phase C: 8 passes x 16 key tiles --------------------------
            avq = []   # (closure) AV work trailing the exp stream
            wbudget = 4000.0  # initial deferred-PE credit (ns)

            def av_mms(p, jt, pbt, av0, av1):
                def f():
                    hp = p // 2
                    h0, h1 = 2 * hp, 2 * hp + 1
                    nc.tensor.matmul(av0[:], Vau[jt][:, h0, 0:65],
                                     pbt[:, 0:512],
                                     start=(jt == 0), stop=(jt == NT - 1))
                    nc.tensor.matmul(av1[:], Vau[jt][:, h1, 0:65],
                                     pbt[:, 512:1024],
                                     start=(jt == 0), stop=(jt == NT - 1))
                return f

            for p in range(8):
                hp, ic = p // 2, p % 2
                av0 = psC.tile([65, 512], F32, tag="av0", bufs=1,
                               name=f"av0_{p}")
                av1 = psC.tile([65, 512], F32, tag="av1", bufs=1,
                               name=f"av1_{p}")
                i5 = ic * 512
                for jt in range(NT):
                    sp = psC.tile([128, 1024], F32, tag="sp", bufs=2)
                    nc.tensor.matmul(
                        sp[:, 0:512],
                        KTp[hp][0:64, jt * 128:(jt + 1) * 128],
                        QTp[hp][0:64, i5:i5 + 512],
                        start=True, stop=True, tile_position=(0, 0))
                    nc.tensor.matmul(
                        sp[:, 512:1024],
                        KTp[hp][64:128, jt * 128:(jt + 1) * 128],
                        QTp[hp][64:128, i5:i5 + 512],
                        start=True, stop=True, tile_position=(64, 0))
                    eb = W.tile([128, 1024], BF16, tag="eb", bufs=3)
                    nc.scalar.activation(out=eb[:], in_=sp[:], func=AF.Exp)
                    pbt = W.tile([128, 1024], BF16, tag="pb", bufs=LAG + 2)
                    ea = expa_q[jt // 4][:, jt % 4, i5:i5 + 512]
                    nc.vector.tensor_mul(out=pbt[:, 0:512], in0=eb[:, 0:512],
                                         in1=ea)
                    nc.vector.tensor_mul(out=pbt[:, 512:1024],
                                         in0=eb[:, 512:1024], in1=ea)
                    avq.append(av_mms(p, jt, pbt, av0, av1))
                    # drain trailing B-work (2/iter keeps producers ahead
                    # of their PE-queue consumers), then lagged AV work
                    # (also at most 2/iter so den roundtrips stay spread).
                    wbudget += 500.0
                    while workq and workq[0][0] <= wbudget:
                        w, fn = workq.pop(0)
                        wbudget -= w
                        fn()
                    for _ in range(2):
                        if len(avq) > LAG:
                            fn = avq.pop(0)
                            if fn is not None:
                                fn()
                # end of pass: trailing AV for this pass still in avq; queue
                # the evacuation work behind them.
                def pass_tail(p=p, av0=av0, av1=av1):
                    hp, ic = p // 2, p % 2
                    h0, h1 = 2 * hp, 2 * hp + 1
                    sl = slice(ic * 512, (ic + 1) * 512)
                    def f():
                        nc.vector.tensor_copy(out=numT[h0][:, sl], in_=av0[:])
                        nc.vector.tensor_copy(out=numT[h1][:, sl], in_=av1[:])
                        if ic == 1:
                            for e in range(2):
                                h = 2 * hp + e
                                nc.scalar.dma_start(out=dsb_d[h, :],
                                                    in_=numT[h][64:65, :])
                    return f
                avq.append(pass_tail())
                if p % 2 == 1:
                    avq.extend(den_pieces(p // 2))

            # flush remaining trailing work, keeping the PE busy through the
            # last denominator roundtrip (dummy matmuls bridge HAM warmth)
            for w, fn in workq:
                fn()
            tail = [fn for fn in avq if fn is not None]
            for i, fn in enumerate(tail):
                fn()
                if i % 2 == 1:
                    dmy = psB.tile([128, 512], F32, tag="bps", bufs=2,
                                   name=f"dmy{i}")
                    nc.tensor.matmul(dmy[:], wqkv_sb[0][:, 0:128],
                                     xt_ch[0][0][:], start=True, stop=True)

        # ---- Phase D: output projection (head pairs, K=128) ----------------
        with tc.tile_pool(name="psD", bufs=1, space="PSUM") as psD:
            yps = [psD.tile([128, 512], F32, tag=f"yp{isl}", name=f"yp{isl}")
                   for isl in range(8)]
            for hp in range(4):
                for isl in range(8):
                    nc.tensor.matmul(yps[isl][:],
                                     pairT[hp][:, isl * 128:(isl + 1) * 128],
                                     woutP[hp][:],
                                     start=(hp == 0), stop=(hp == 3))
            for isl in range(8):
                ysb = W.tile([128, 512], F32, tag="ysb", bufs=2)
                if has_b:
                    nc.vector.tensor_add(out=ysb[:], in0=yps[isl][:],
                                         in1=bout_bc[:])
                elif isl % 2 == 0:
                    nc.vector.tensor_copy(out=ysb[:], in_=yps[isl][:])
                else:
                    nc.scalar.copy(out=ysb[:], in_=yps[isl][:])
                nc.sync.dma_start(out=y_d[isl * 128:(isl + 1) * 128, :],
                                  in_=ysb[:])
    if split_waits:
        _split_waits(nc)
    return nc


_NC_CACHE = {}


def _get_nc(has_c, has_b):
    key = (has_c, has_b)
    if key not in _NC_CACHE:
        _NC_CACHE[key] = build(has_c, has_b)
    return _NC_CACHE[key]


LAST_EXEC_TIME_NS = None


def kernel(x, gamma, beta, Wqkv, Wout, bout, rel_table, temporal_mask,
           trace=True):
    global LAST_EXEC_TIME_NS
    x = np.asarray(x, np.float32)
    gamma = np.asarray(gamma, np.float32)
    beta = np.asarray(beta, np.float32)
    Wqkv = np.asarray(Wqkv, np.float32)
    Wout = np.asarray(Wout, np.float32)
    bout = np.asarray(bout, np.float32)
    rel_table = np.asarray(rel_table, np.float32)
    temporal_mask = np.asarray(temporal_mask)

    scale = DH ** -0.5
    w_eff = (Wqkv * gamma[:, None]).copy()
    w_eff[:, :D] *= scale
    cqkv = (beta @ Wqkv).astype(np.float32)
    cqkv[:D] *= scale
    # column reorder: [K0 Q0 K1 Q1 K2 Q2 K3 Q3 V] so hp0's K/Q land first
    perm = []
    for hp in range(H // 2):
        perm += list(range(D + hp * 128, D + (hp + 1) * 128))
        perm += list(range(hp * 128, (hp + 1) * 128))
    perm += list(range(2 * D, 3 * D))
    w_eff = np.ascontiguousarray(w_eff[:, perm])
    cqkv = np.ascontiguousarray(cqkv[perm])
    wqkv_bf = w_eff.astype(ml_dtypes.bfloat16)
    # rank-1 mean correction: -g rows (so the matmul accumulates -mu*g)
    grow_bf = (-w_eff.sum(axis=0)).reshape(1, -1).astype(ml_dtypes.bfloat16)
    wout_bf = Wout.astype(ml_dtypes.bfloat16)
    mask01 = (temporal_mask != 0)

    idx = np.arange(N)
    # expbias[i, j] = exp(rel_table[clip(i - j)]) with i=query, j=key
    expbias = np.exp(rel_table[
        np.clip(idx[:, None] - idx[None, :], -(MAXREL - 1), MAXREL - 1)
        + MAXREL - 1]).astype(np.float32)

    keyperm_half = [
        np.concatenate([np.arange(i0, i0 + NQ),
                        np.arange(NQ - i0, NQ - i0 + NQ)])
        for i0 in (0, NQ)
    ]
    # expA[j_perm, i_local] = exp(bias(query i, key j)) * mask(query i, key j)
    expa_half = []
    for half in range(2):
        kp = keyperm_half[half]
        qs = np.arange(half * NQ, (half + 1) * NQ)
        a = (expbias[np.ix_(qs, kp)] * mask01[np.ix_(qs, kp)]).T
        expa_half.append(np.ascontiguousarray(a).astype(ml_dtypes.bfloat16))

    in_maps = []
    for c in range(NCORES):
        b, half = c // 2, c % 2
        xp = np.ascontiguousarray(x[b][keyperm_half[half]])
        xtp = np.ascontiguousarray(xp.T).astype(ml_dtypes.bfloat16)
        in_maps.append({
            "x": xp.astype(ml_dtypes.bfloat16),
            "xt": xtp,
            "wqkv": wqkv_bf,
            "grow": grow_bf,
            "cqkv": cqkv,
            "wout": wout_bf,
            "bout": bout,
            "expa": expa_half[half],
        })

    nc = _get_nc(bool(np.any(cqkv != 0.0)), bool(np.any(bout != 0.0)))
    res = run_bass_kernel_spmd(nc, in_maps, core_ids=list(range(NCORES)),
                               trace=trace)
    LAST_EXEC_TIME_NS = res.exec_time_ns

    out = np.empty((B, N, D), np.float32)
    for c in range(NCORES):
        b, half = c // 2, c % 2
        out[b, half * NQ:(half + 1) * NQ] = res.results[c]["y"]
    return out
